# revision 2
# baseline (speedup 1.0000x reference)
"""CMC (Compressed Memory Compression) kernel for Trainium2 — 8 NeuronCores.

Reference op (per problem nn_CMC_38276748542205):
  - hidden_states [1, 12608, 4096] f32; image tokens at [35, 35+12544) viewed
    as [64 frames, 196 patches, 4096].
  - Frames form 16 intervals of 4; I-frame at position 3 of each interval.
  - SAD(token, I-frame token at same patch) over dim; mask = SAD < 1.12*4096.
  - Masked tokens replaced by the interval's I-frame token.

Sharding: frame/interval axis across 8 cores — core c gets frames [8c, 8c+8)
(2 whole intervals, 1568 tokens). Text tokens (64 rows) pass through on host.

Device kernel (per core, SPMD):
  - patch-major tiles [P<=128 patches, 4 frames, 4096] via strided DMA; the
    I-frame is the f=3 slice of the same tile (no extra traffic).
  - DVE: d_k = p3 - p_k (k in 0..2; f=3 output is identity, skipped).
  - ACT: |d_k| with per-2048-chunk accumulation -> SAD (chunked to keep fp32
    summation error well below the min |SAD-thr| margin of ~0.034).
  - DVE: mask m (is_lt), c = 1-m, diagonal matrices diag(m), diag(c) built
    from a constant eye via per-partition tensor_scalar.
  - PE:  psum = diag(c) @ p_k + diag(m) @ p3  (the select/blend as matmuls).
  - ACT: copy psum back over p_k in SBUF (in place), then DMA the whole
    [P, 4, 4096] tile out.
"""

import functools

import numpy as np

# ---- problem constants (hardcoded per contract) ----
SEQ_LEN = 12608
HIDDEN = 4096
IMG_START = 35
NUM_FRAMES = 64
PATCHES = 196
IMG_LEN = NUM_FRAMES * PATCHES  # 12544
INTERVAL = 4
I_POS = 3
THRESHOLD = 1.12 * HIDDEN  # 4587.52

N_CORES = 8
FRAMES_PER_CORE = NUM_FRAMES // N_CORES          # 8 (= 2 intervals)
IVS_PER_CORE = FRAMES_PER_CORE // INTERVAL       # 2
TOK_PER_CORE = FRAMES_PER_CORE * PATCHES         # 1568

USE_F32R = False       # float32r is reduced-precision (TF32-like); keep fp32
SAD_CHUNK = 2048       # accumulation chunk for SAD numerical accuracy
MM_N = 512             # matmul moving-dim chunk (<= 512 for fp32 family)
PSUM_FREE = 2048       # psum tile free size (4 banks)


def _kernel_body(tc, y_ap, x_ap, eye_ap):
    import concourse.bass as bass
    from concourse import mybir

    nc = tc.nc
    AF = mybir.ActivationFunctionType
    OP = mybir.AluOpType
    f32 = mybir.dt.float32

    def mmcast(ap):
        return ap.bitcast(mybir.dt.float32r) if USE_F32R else ap

    xv = x_ap.rearrange("(f p) d -> p f d", f=FRAMES_PER_CORE, p=PATCHES)
    yv = y_ap.rearrange("(f p) d -> p f d", f=FRAMES_PER_CORE, p=PATCHES)

    import contextlib

    with contextlib.ExitStack() as ctx:
        const_pool = ctx.enter_context(tc.tile_pool(name="const", bufs=1))
        p_pool = ctx.enter_context(tc.tile_pool(name="p", bufs=2))
        d_pool = ctx.enter_context(tc.tile_pool(name="d", bufs=2))
        abs_pool = ctx.enter_context(tc.tile_pool(name="absd", bufs=2))
        small_pool = ctx.enter_context(tc.tile_pool(name="small", bufs=12))
        diag_pool = ctx.enter_context(tc.tile_pool(name="diag", bufs=4))
        psum_pool = ctx.enter_context(
            tc.tile_pool(name="ps", bufs=2, space="PSUM")
        )

        eye_t = const_pool.tile([128, 128], f32)
        nc.sync.dma_start(eye_t[:, :], eye_ap[:, :])

        n_sad_chunks = HIDDEN // SAD_CHUNK

        for iv in range(IVS_PER_CORE):
            for pc, P in ((0, 128), (1, PATCHES - 128)):
                p0 = pc * 128
                f0 = iv * INTERVAL
                pt = p_pool.tile([128, INTERVAL, HIDDEN], f32)
                nc.sync.dma_start(
                    pt[:P, :, :], xv[p0 : p0 + P, f0 : f0 + INTERVAL, :]
                )
                for k in range(INTERVAL - 1):  # f=3 (I-frame) passes through
                    d_t = d_pool.tile([128, HIDDEN], f32)
                    nc.vector.tensor_tensor(
                        d_t[:P, :], pt[:P, I_POS, :], pt[:P, k, :], op=OP.subtract
                    )
                    sadp = small_pool.tile([128, n_sad_chunks], f32, tag="sadp")
                    for h in range(n_sad_chunks):
                        ab = abs_pool.tile([128, SAD_CHUNK], f32)
                        nc.scalar.activation(
                            ab[:P, :],
                            d_t[:P, bass.ts(h, SAD_CHUNK)],
                            AF.Abs,
                            accum_out=sadp[:P, h : h + 1],
                        )
                    sad = small_pool.tile([128, 1], f32, tag="sad")
                    nc.vector.tensor_reduce(
                        sad[:P, :], sadp[:P, :], axis=mybir.AxisListType.X, op=OP.add
                    )
                    m_t = small_pool.tile([128, 1], f32, tag="m")
                    nc.vector.tensor_scalar(
                        m_t[:P, :], sad[:P, :], float(THRESHOLD), None, op0=OP.is_lt
                    )
                    c_t = small_pool.tile([128, 1], f32, tag="c")
                    nc.vector.tensor_scalar(
                        c_t[:P, :], m_t[:P, :], -1.0, 1.0, op0=OP.mult, op1=OP.add
                    )
                    dg_m = diag_pool.tile([128, 128], f32, tag="dgm")
                    nc.vector.tensor_scalar(
                        dg_m[:P, :], eye_t[:P, :], m_t[:P, :], None, op0=OP.mult
                    )
                    dg_c = diag_pool.tile([128, 128], f32, tag="dgc")
                    nc.vector.tensor_scalar(
                        dg_c[:P, :], eye_t[:P, :], c_t[:P, :], None, op0=OP.mult
                    )
                    for half in range(HIDDEN // PSUM_FREE):
                        ps = psum_pool.tile([128, PSUM_FREE], f32)
                        for j in range(PSUM_FREE // MM_N):
                            n0 = half * PSUM_FREE + j * MM_N
                            nc.tensor.matmul(
                                ps[:, bass.ts(j, MM_N)],
                                mmcast(dg_c[:P, :]),
                                mmcast(pt[:P, k, n0 : n0 + MM_N]),
                                start=True,
                                stop=False,
                            )
                            nc.tensor.matmul(
                                ps[:, bass.ts(j, MM_N)],
                                mmcast(dg_m[:P, :]),
                                mmcast(pt[:P, I_POS, n0 : n0 + MM_N]),
                                start=False,
                                stop=True,
                            )
                        nc.scalar.copy(
                            pt[:P, k, bass.ts(half, PSUM_FREE)], ps[:P, :]
                        )
                nc.sync.dma_start(
                    yv[p0 : p0 + P, f0 : f0 + INTERVAL, :], pt[:P, :, :]
                )


@functools.cache
def _build_nc():
    import concourse.bacc as bacc
    import concourse.tile as tile
    from concourse import mybir

    nc = bacc.Bacc(
        "TRN2",
        target_bir_lowering=False,
        debug=False,
        enable_asserts=False,
        num_devices=N_CORES,
    )
    x = nc.dram_tensor(
        "x", [TOK_PER_CORE, HIDDEN], mybir.dt.float32, kind="ExternalInput"
    ).ap()
    eye = nc.dram_tensor(
        "eye", [128, 128], mybir.dt.float32, kind="ExternalInput"
    ).ap()
    y = nc.dram_tensor(
        "y", [TOK_PER_CORE, HIDDEN], mybir.dt.float32, kind="ExternalOutput"
    ).ap()
    with tile.TileContext(nc) as tc:
        _kernel_body(tc, y, x, eye)
    nc.compile()
    return nc


def _in_maps(hs: np.ndarray):
    img = hs[0, IMG_START : IMG_START + IMG_LEN]
    eye = np.eye(128, dtype=np.float32)
    maps = []
    for c in range(N_CORES):
        xc = img[TOK_PER_CORE * c : TOK_PER_CORE * (c + 1)]
        maps.append({"x": np.ascontiguousarray(xc), "eye": eye})
    return maps


def kernel(hidden_states: np.ndarray) -> np.ndarray:
    from concourse.bass_utils import run_bass_kernel_spmd

    hs = np.asarray(hidden_states, dtype=np.float32)
    assert hs.shape == (1, SEQ_LEN, HIDDEN), hs.shape
    nc = _build_nc()
    res = run_bass_kernel_spmd(nc, _in_maps(hs), list(range(N_CORES)))
    out = hs.copy()
    for c in range(N_CORES):
        out[0, IMG_START + TOK_PER_CORE * c : IMG_START + TOK_PER_CORE * (c + 1)] = (
            res.results[c]["y"]
        )
    return out


# revision 4
# speedup vs baseline: 1.6248x; 1.6248x over previous
"""CMC (Compressed Memory Compression) kernel for Trainium2 — 8 NeuronCores.

Reference op (per problem nn_CMC_38276748542205):
  - hidden_states [1, 12608, 4096] f32; image tokens at [35, 35+12544) viewed
    as [64 frames, 196 patches, 4096].
  - Frames form 16 intervals of 4; I-frame at position 3 of each interval.
  - SAD(token, I-frame token at same patch) over dim; mask = SAD < 1.12*4096.
  - Masked tokens replaced by the interval's I-frame token.

Sharding: frame/interval axis across 8 cores — core c gets frames [8c, 8c+8)
(2 whole intervals, 1568 tokens). Text tokens (64 rows) pass through on host.

Device kernel (per core, SPMD):
  - patch-major tiles [P<=128 patches, 4 frames, 4096] via strided DMA; the
    I-frame is the f=3 slice of the same tile (no extra traffic).
  - DVE: d_k = p3 - p_k (k in 0..2; f=3 output is identity, skipped).
  - ACT: |d_k| with per-2048-chunk accumulation -> SAD (chunked to keep fp32
    summation error well below the min |SAD-thr| margin of ~0.034).
  - DVE: mask m (is_lt), c = 1-m, diagonal matrices diag(m), diag(c) built
    from a constant eye via per-partition tensor_scalar.
  - PE:  psum = diag(c) @ p_k + diag(m) @ p3  (the select/blend as matmuls).
  - ACT: copy psum back over p_k in SBUF (in place), then DMA the whole
    [P, 4, 4096] tile out.
"""

import functools

import numpy as np

# ---- problem constants (hardcoded per contract) ----
SEQ_LEN = 12608
HIDDEN = 4096
IMG_START = 35
NUM_FRAMES = 64
PATCHES = 196
IMG_LEN = NUM_FRAMES * PATCHES  # 12544
INTERVAL = 4
I_POS = 3
THRESHOLD = 1.12 * HIDDEN  # 4587.52

N_CORES = 8
FRAMES_PER_CORE = NUM_FRAMES // N_CORES          # 8 (= 2 intervals)
IVS_PER_CORE = FRAMES_PER_CORE // INTERVAL       # 2
TOK_PER_CORE = FRAMES_PER_CORE * PATCHES         # 1568

SAD_CHUNK = 2048       # accumulation chunk for SAD numerical accuracy


def _kernel_body(tc, y_ap, x_ap):
    import concourse.bass as bass
    from concourse import mybir

    nc = tc.nc
    AF = mybir.ActivationFunctionType
    OP = mybir.AluOpType
    f32 = mybir.dt.float32

    xv = x_ap.rearrange("(f p) d -> p f d", f=FRAMES_PER_CORE, p=PATCHES)
    yv = y_ap.rearrange("(f p) d -> p f d", f=FRAMES_PER_CORE, p=PATCHES)

    import contextlib

    with contextlib.ExitStack() as ctx:
        p_pool = ctx.enter_context(tc.tile_pool(name="p", bufs=2))
        d_pool = ctx.enter_context(tc.tile_pool(name="d", bufs=3))
        abs_pool = ctx.enter_context(tc.tile_pool(name="absd", bufs=2))
        small_pool = ctx.enter_context(tc.tile_pool(name="small", bufs=12))

        n_sad_chunks = HIDDEN // SAD_CHUNK

        for iv in range(IVS_PER_CORE):
            for pc, P in ((0, 128), (1, PATCHES - 128)):
                p0 = pc * 128
                f0 = iv * INTERVAL
                pt = p_pool.tile([128, INTERVAL, HIDDEN], f32)
                # per-frame loads: I-frame first so subs can start early
                for f in (I_POS, 0, 1, 2):
                    nc.sync.dma_start(
                        pt[:P, f, :], xv[p0 : p0 + P, f0 + f, :]
                    )
                # I-frame output is the identity: ship it out immediately
                nc.sync.dma_start(yv[p0 : p0 + P, f0 + I_POS, :], pt[:P, I_POS, :])
                for k in range(INTERVAL - 1):  # f=3 (I-frame) passes through
                    d_t = d_pool.tile([128, HIDDEN], f32)
                    nc.vector.tensor_tensor(
                        d_t[:P, :], pt[:P, I_POS, :], pt[:P, k, :], op=OP.subtract
                    )
                    sadp = small_pool.tile([128, n_sad_chunks], f32, tag="sadp")
                    for h in range(n_sad_chunks):
                        ab = abs_pool.tile([128, SAD_CHUNK], f32)
                        nc.scalar.activation(
                            ab[:P, :],
                            d_t[:P, bass.ts(h, SAD_CHUNK)],
                            AF.Abs,
                            accum_out=sadp[:P, h : h + 1],
                        )
                    m_t = small_pool.tile([128, 1], f32, tag="m")
                    # sum the per-chunk SADs, then threshold: m = sad < thr
                    sad = small_pool.tile([128, 1], f32, tag="sad")
                    nc.vector.tensor_reduce(
                        sad[:P, :], sadp[:P, :], axis=mybir.AxisListType.X, op=OP.add
                    )
                    nc.vector.tensor_scalar(
                        m_t[:P, :], sad[:P, :], float(THRESHOLD), None, op0=OP.is_lt
                    )
                    # fused blend: out = (d * m) + p_k, in place over p_k
                    nc.vector.scalar_tensor_tensor(
                        pt[:P, k, :],
                        d_t[:P, :],
                        m_t[:P, :],
                        pt[:P, k, :],
                        op0=OP.mult,
                        op1=OP.add,
                    )
                    nc.sync.dma_start(yv[p0 : p0 + P, f0 + k, :], pt[:P, k, :])


@functools.cache
def _build_nc():
    import concourse.bacc as bacc
    import concourse.tile as tile
    from concourse import mybir

    nc = bacc.Bacc(
        "TRN2",
        target_bir_lowering=False,
        debug=False,
        enable_asserts=False,
        num_devices=N_CORES,
    )
    x = nc.dram_tensor(
        "x", [TOK_PER_CORE, HIDDEN], mybir.dt.float32, kind="ExternalInput"
    ).ap()
    y = nc.dram_tensor(
        "y", [TOK_PER_CORE, HIDDEN], mybir.dt.float32, kind="ExternalOutput"
    ).ap()
    with tile.TileContext(nc) as tc:
        _kernel_body(tc, y, x)
    nc.compile()
    return nc


def _in_maps(hs: np.ndarray):
    img = hs[0, IMG_START : IMG_START + IMG_LEN]
    maps = []
    for c in range(N_CORES):
        xc = img[TOK_PER_CORE * c : TOK_PER_CORE * (c + 1)]
        maps.append({"x": np.ascontiguousarray(xc)})
    return maps


def kernel(hidden_states: np.ndarray) -> np.ndarray:
    from concourse.bass_utils import run_bass_kernel_spmd

    hs = np.asarray(hidden_states, dtype=np.float32)
    assert hs.shape == (1, SEQ_LEN, HIDDEN), hs.shape
    nc = _build_nc()
    res = run_bass_kernel_spmd(nc, _in_maps(hs), list(range(N_CORES)))
    out = hs.copy()
    for c in range(N_CORES):
        out[0, IMG_START + TOK_PER_CORE * c : IMG_START + TOK_PER_CORE * (c + 1)] = (
            res.results[c]["y"]
        )
    return out


# revision 24
# speedup vs baseline: 2.5453x; 1.5665x over previous
"""CMC (Compressed Memory Compression) kernel for Trainium2 — 8 NeuronCores.

Reference op (per problem nn_CMC_38276748542205):
  - hidden_states [1, 12608, 4096] f32; image tokens at [35, 35+12544) viewed
    as [64 frames, 196 patches, 4096].
  - Frames form 16 intervals of 4; I-frame at position 3 of each interval.
  - SAD(token, I-frame token at same patch) over dim; mask = SAD < 1.12*4096.
  - Masked tokens replaced by the interval's I-frame token.

Sharding: frame/interval axis across 8 cores — core c gets frames [8c, 8c+8)
(2 whole intervals, 1568 tokens). Text tokens (64 rows) pass through on host.

Device kernel (per core, SPMD):
  - patch-major tiles [128 patches, 4 frames, 4096] via strided DMA; the
    I-frame is the f=3 slice of the same tile (no extra traffic, perfect
    partition alignment for the per-patch compare).
  - DVE: d_k = p3 - p_k (k in {0,1,2}; the f=3 output is the identity).
  - ACT: |d_k| with per-2048-chunk accumulation -> SAD (chunked so fp32
    summation error stays well below the min |SAD-thr| margin of ~0.034).
  - DVE: m = (sad < thr) as a per-partition 0/1 scalar, then the whole
    select/replace is ONE fused DVE op (scalar_tensor_tensor):
    out = (d * m) + p_k, written in place over p_k; 2-frame half-stores
    drain as soon as their frames are blended.
"""

import functools

import numpy as np

# ---- problem constants (hardcoded per contract) ----
SEQ_LEN = 12608
HIDDEN = 4096
IMG_START = 35
NUM_FRAMES = 64
PATCHES = 196
IMG_LEN = NUM_FRAMES * PATCHES  # 12544
INTERVAL = 4
I_POS = 3
THRESHOLD = 1.12 * HIDDEN  # 4587.52

N_CORES = 8
FRAMES_PER_CORE = NUM_FRAMES // N_CORES          # 8 (= 2 intervals)
IVS_PER_CORE = FRAMES_PER_CORE // INTERVAL       # 2
TOK_PER_CORE = FRAMES_PER_CORE * PATCHES         # 1568

SAD_CHUNK = 2048       # accumulation chunk for SAD numerical accuracy


def _kernel_body(tc, y_ap, x_ap):
    import concourse.bass as bass
    from concourse import mybir

    nc = tc.nc
    AF = mybir.ActivationFunctionType
    OP = mybir.AluOpType
    f32 = mybir.dt.float32

    xv = x_ap.rearrange("(f p) d -> p f d", f=FRAMES_PER_CORE, p=PATCHES)
    yv = y_ap.rearrange("(f p) d -> p f d", f=FRAMES_PER_CORE, p=PATCHES)

    import contextlib

    with contextlib.ExitStack() as ctx:
        p_pool = ctx.enter_context(tc.tile_pool(name="p", bufs=2))
        d_pool = ctx.enter_context(tc.tile_pool(name="d", bufs=3))
        abs_pool = ctx.enter_context(tc.tile_pool(name="absd", bufs=2))
        small_pool = ctx.enter_context(tc.tile_pool(name="small", bufs=12))

        n_sad_chunks = HIDDEN // SAD_CHUNK

        # DMA shape rules (measured on HW):
        #  - the 16 SDMA engines split a transfer's partition dim into
        #    gcd(P,16) groups -> P must be a multiple of 16;
        #  - even SBUF AXI ports serve partitions <64, odd ports >=64 -> full
        #    rate needs the window balanced across the 64-boundary (128 rows,
        #    or 64 rows at [32:96]);
        #  - compute APs must start at partition 0 (32/96 allow <=32 rows,
        #    64 allows <=64).
        # Patch coverage: chunk A = patches 0-127 at [0:128]; chunk B =
        # patches 128-191 at partitions [32:96] (compute on [0:96], the
        # garbage rows [0:32) are zeroed once and never stored). Patches
        # 192-195 (the %16 runt) are handled host-side in numpy.
        def compute_and_store(pt, q1, store):
            for k in (2, 0, 1):  # f=3 (I-frame) passes through untouched
                d_t = d_pool.tile([128, HIDDEN], f32)
                nc.vector.tensor_tensor(
                    d_t[:q1, :],
                    pt[:q1, I_POS, :],
                    pt[:q1, k, :],
                    op=OP.subtract,
                )
                sadp = small_pool.tile([128, n_sad_chunks], f32, tag="sadp")
                for h in range(n_sad_chunks):
                    ab = abs_pool.tile([128, SAD_CHUNK], f32)
                    nc.scalar.activation(
                        ab[:q1, :],
                        d_t[:q1, bass.ts(h, SAD_CHUNK)],
                        AF.Abs,
                        accum_out=sadp[:q1, h : h + 1],
                    )
                m_t = small_pool.tile([128, 1], f32, tag="m")
                # sum the per-chunk SADs, then threshold: m = sad < thr
                sad = small_pool.tile([128, 1], f32, tag="sad")
                nc.vector.tensor_reduce(
                    sad[:q1, :],
                    sadp[:q1, :],
                    axis=mybir.AxisListType.X,
                    op=OP.add,
                )
                nc.vector.tensor_scalar(
                    m_t[:q1, :], sad[:q1, :], float(THRESHOLD), None,
                    op0=OP.is_lt,
                )
                # fused blend: out = (d * m) + p_k, in place over p_k
                nc.vector.scalar_tensor_tensor(
                    pt[:q1, k, :],
                    d_t[:q1, :],
                    m_t[:q1, :],
                    pt[:q1, k, :],
                    op0=OP.mult,
                    op1=OP.add,
                )
                if k == 2:
                    store(2, INTERVAL)  # frames 2-3 ready: drain early
            store(0, 2)

        for iv in range(IVS_PER_CORE):
            f0 = iv * INTERVAL

            # ---- chunk A: patches 0-127 at [0:128] ----
            ptA = p_pool.tile([128, INTERVAL, HIDDEN], f32, tag="pt")
            # paired loads, I-frame half first (sub k=2 needs only f2/f3)
            nc.sync.dma_start(ptA[:, 2:4, :], xv[0:128, f0 + 2 : f0 + 4, :])
            nc.sync.dma_start(ptA[:, 0:2, :], xv[0:128, f0 : f0 + 2, :])

            def store_a(fa, fb, ptA=ptA, f0=f0):
                nc.sync.dma_start(
                    yv[0:128, f0 + fa : f0 + fb, :], ptA[:, fa:fb, :]
                )

            compute_and_store(ptA, 128, store_a)

            # ---- chunk B: patches 128-191 at partitions [32:96] ----
            ptB = p_pool.tile([128, INTERVAL, HIDDEN], f32, tag="pt")
            # rows [0:32) are read by the [0:96] compute ops but never
            # loaded; zero them (gpsimd, off the critical engines)
            nc.gpsimd.memset(ptB[0:32, :, :], 0.0)
            nc.sync.dma_start(
                ptB[32:96, 2:4, :], xv[128:192, f0 + 2 : f0 + 4, :]
            )
            nc.sync.dma_start(ptB[32:96, 0:2, :], xv[128:192, f0 : f0 + 2, :])

            def store_b(fa, fb, ptB=ptB, f0=f0):
                nc.sync.dma_start(
                    yv[128:192, f0 + fa : f0 + fb, :], ptB[32:96, fa:fb, :]
                )

            compute_and_store(ptB, 96, store_b)


@functools.cache
def _build_nc():
    import concourse.bacc as bacc
    import concourse.tile as tile
    from concourse import mybir

    nc = bacc.Bacc(
        "TRN2",
        target_bir_lowering=False,
        debug=False,
        enable_asserts=False,
        num_devices=N_CORES,
    )
    x = nc.dram_tensor(
        "x", [TOK_PER_CORE, HIDDEN], mybir.dt.float32, kind="ExternalInput"
    ).ap()
    y = nc.dram_tensor(
        "y", [TOK_PER_CORE, HIDDEN], mybir.dt.float32, kind="ExternalOutput"
    ).ap()
    with tile.TileContext(nc) as tc:
        _kernel_body(tc, y, x)
    nc.compile()
    return nc


def _in_maps(hs: np.ndarray):
    img = hs[0, IMG_START : IMG_START + IMG_LEN]
    maps = []
    for c in range(N_CORES):
        xc = img[TOK_PER_CORE * c : TOK_PER_CORE * (c + 1)]
        maps.append({"x": np.ascontiguousarray(xc)})
    return maps


def _host_runt(img: np.ndarray) -> np.ndarray:
    """Blend for patches 192-195 (the %16 runt the device skips): numpy."""
    iv = img.reshape(NUM_FRAMES // INTERVAL, INTERVAL, PATCHES, HIDDEN)
    runt = iv[:, :, 192:PATCHES, :]
    itok = runt[:, I_POS : I_POS + 1]
    d = itok.astype(np.float64) - runt.astype(np.float64)
    mask = np.abs(d).sum(-1) < THRESHOLD
    return np.where(mask[..., None], itok, runt).astype(np.float32)


def kernel(hidden_states: np.ndarray) -> np.ndarray:
    from concourse.bass_utils import run_bass_kernel_spmd

    hs = np.asarray(hidden_states, dtype=np.float32)
    assert hs.shape == (1, SEQ_LEN, HIDDEN), hs.shape
    nc = _build_nc()
    res = run_bass_kernel_spmd(nc, _in_maps(hs), list(range(N_CORES)))
    out = hs.copy()
    for c in range(N_CORES):
        out[0, IMG_START + TOK_PER_CORE * c : IMG_START + TOK_PER_CORE * (c + 1)] = (
            res.results[c]["y"]
        )
    img = hs[0, IMG_START : IMG_START + IMG_LEN]
    outv = out[0, IMG_START : IMG_START + IMG_LEN].reshape(
        NUM_FRAMES, PATCHES, HIDDEN
    )
    outv[:, 192:PATCHES, :] = _host_runt(img).reshape(NUM_FRAMES, 4, HIDDEN)
    return out


# revision 25
# speedup vs baseline: 2.5674x; 1.0087x over previous
"""CMC (Compressed Memory Compression) kernel for Trainium2 — 8 NeuronCores.

Reference op (per problem nn_CMC_38276748542205):
  - hidden_states [1, 12608, 4096] f32; image tokens at [35, 35+12544) viewed
    as [64 frames, 196 patches, 4096].
  - Frames form 16 intervals of 4; I-frame at position 3 of each interval.
  - SAD(token, I-frame token at same patch) over dim; mask = SAD < 1.12*4096.
  - Masked tokens replaced by the interval's I-frame token.

Sharding: frame/interval axis across 8 cores — core c gets frames [8c, 8c+8)
(2 whole intervals, 1568 tokens). Text tokens (64 rows) pass through on host.

Device kernel (per core, SPMD):
  - patch-major tiles [128 patches, 4 frames, 4096] via strided DMA; the
    I-frame is the f=3 slice of the same tile (no extra traffic, perfect
    partition alignment for the per-patch compare).
  - DVE: d_k = p3 - p_k (k in {0,1,2}; the f=3 output is the identity).
  - ACT: |d_k| with per-2048-chunk accumulation -> SAD (chunked so fp32
    summation error stays well below the min |SAD-thr| margin of ~0.034).
  - DVE: m = (sad < thr) as a per-partition 0/1 scalar, then the whole
    select/replace is ONE fused DVE op (scalar_tensor_tensor):
    out = (d * m) + p_k, written in place over p_k; 2-frame half-stores
    drain as soon as their frames are blended.
"""

import functools

import numpy as np

# ---- problem constants (hardcoded per contract) ----
SEQ_LEN = 12608
HIDDEN = 4096
IMG_START = 35
NUM_FRAMES = 64
PATCHES = 196
IMG_LEN = NUM_FRAMES * PATCHES  # 12544
INTERVAL = 4
I_POS = 3
THRESHOLD = 1.12 * HIDDEN  # 4587.52

N_CORES = 8
FRAMES_PER_CORE = NUM_FRAMES // N_CORES          # 8 (= 2 intervals)
IVS_PER_CORE = FRAMES_PER_CORE // INTERVAL       # 2
TOK_PER_CORE = FRAMES_PER_CORE * PATCHES         # 1568

SAD_CHUNK = 2048       # accumulation chunk for SAD numerical accuracy


def _kernel_body(tc, y_ap, x_ap):
    import concourse.bass as bass
    from concourse import mybir

    nc = tc.nc
    AF = mybir.ActivationFunctionType
    OP = mybir.AluOpType
    f32 = mybir.dt.float32

    xv = x_ap.rearrange("(f p) d -> p f d", f=FRAMES_PER_CORE, p=PATCHES)
    yv = y_ap.rearrange("(f p) d -> p f d", f=FRAMES_PER_CORE, p=PATCHES)

    import contextlib

    with contextlib.ExitStack() as ctx:
        p_pool = ctx.enter_context(tc.tile_pool(name="p", bufs=2))
        d_pool = ctx.enter_context(tc.tile_pool(name="d", bufs=3))
        abs_pool = ctx.enter_context(tc.tile_pool(name="absd", bufs=2))
        small_pool = ctx.enter_context(tc.tile_pool(name="small", bufs=12))

        n_sad_chunks = HIDDEN // SAD_CHUNK

        # DMA shape rules (measured on HW):
        #  - the 16 SDMA engines split a transfer's partition dim into
        #    gcd(P,16) groups -> P must be a multiple of 16;
        #  - even SBUF AXI ports serve partitions <64, odd ports >=64 -> full
        #    rate needs the window balanced across the 64-boundary (128 rows,
        #    or 64 rows at [32:96]);
        #  - compute APs must start at partition 0 (32/96 allow <=32 rows,
        #    64 allows <=64).
        # Patch coverage: chunk A = patches 0-127 at [0:128]; chunk B =
        # patches 128-191 at partitions [32:96] (compute on [0:96], the
        # garbage rows [0:32) are zeroed once and never stored). Patches
        # 192-195 (the %16 runt) are handled host-side in numpy.
        def compute_and_store(pt, q1, store):
            for k in (2, 0, 1):  # f=3 (I-frame) passes through untouched
                d_t = d_pool.tile([128, HIDDEN], f32)
                nc.vector.tensor_tensor(
                    d_t[:q1, :],
                    pt[:q1, I_POS, :],
                    pt[:q1, k, :],
                    op=OP.subtract,
                )
                sadp = small_pool.tile([128, n_sad_chunks], f32, tag="sadp")
                for h in range(n_sad_chunks):
                    ab = abs_pool.tile([128, SAD_CHUNK], f32)
                    nc.scalar.activation(
                        ab[:q1, :],
                        d_t[:q1, bass.ts(h, SAD_CHUNK)],
                        AF.Abs,
                        accum_out=sadp[:q1, h : h + 1],
                    )
                m_t = small_pool.tile([128, 1], f32, tag="m")
                # fused: m = (sadp0 + sadp1) < thr — both scalars per-partition
                nc.vector.tensor_scalar(
                    m_t[:q1, :],
                    sadp[:q1, 0:1],
                    sadp[:q1, 1:2],
                    float(THRESHOLD),
                    op0=OP.add,
                    op1=OP.is_lt,
                )
                # fused blend: out = (d * m) + p_k, in place over p_k
                nc.vector.scalar_tensor_tensor(
                    pt[:q1, k, :],
                    d_t[:q1, :],
                    m_t[:q1, :],
                    pt[:q1, k, :],
                    op0=OP.mult,
                    op1=OP.add,
                )
                if k == 2:
                    store(2, INTERVAL)  # frames 2-3 ready: drain early
            store(0, 2)

        for iv in range(IVS_PER_CORE):
            f0 = iv * INTERVAL

            # ---- chunk A: patches 0-127 at [0:128] ----
            ptA = p_pool.tile([128, INTERVAL, HIDDEN], f32, tag="pt")
            # paired loads, I-frame half first (sub k=2 needs only f2/f3)
            nc.sync.dma_start(ptA[:, 2:4, :], xv[0:128, f0 + 2 : f0 + 4, :])
            nc.sync.dma_start(ptA[:, 0:2, :], xv[0:128, f0 : f0 + 2, :])

            def store_a(fa, fb, ptA=ptA, f0=f0):
                nc.sync.dma_start(
                    yv[0:128, f0 + fa : f0 + fb, :], ptA[:, fa:fb, :]
                )

            compute_and_store(ptA, 128, store_a)

            # ---- chunk B: patches 128-191 at partitions [32:96] ----
            ptB = p_pool.tile([128, INTERVAL, HIDDEN], f32, tag="pt")
            # rows [0:32) are read by the [0:96] compute ops but never
            # loaded; zero them (gpsimd, off the critical engines)
            nc.gpsimd.memset(ptB[0:32, :, :], 0.0)
            nc.sync.dma_start(
                ptB[32:96, 2:4, :], xv[128:192, f0 + 2 : f0 + 4, :]
            )
            nc.sync.dma_start(ptB[32:96, 0:2, :], xv[128:192, f0 : f0 + 2, :])

            def store_b(fa, fb, ptB=ptB, f0=f0):
                nc.sync.dma_start(
                    yv[128:192, f0 + fa : f0 + fb, :], ptB[32:96, fa:fb, :]
                )

            compute_and_store(ptB, 96, store_b)


@functools.cache
def _build_nc():
    import concourse.bacc as bacc
    import concourse.tile as tile
    from concourse import mybir

    nc = bacc.Bacc(
        "TRN2",
        target_bir_lowering=False,
        debug=False,
        enable_asserts=False,
        num_devices=N_CORES,
    )
    x = nc.dram_tensor(
        "x", [TOK_PER_CORE, HIDDEN], mybir.dt.float32, kind="ExternalInput"
    ).ap()
    y = nc.dram_tensor(
        "y", [TOK_PER_CORE, HIDDEN], mybir.dt.float32, kind="ExternalOutput"
    ).ap()
    with tile.TileContext(nc) as tc:
        _kernel_body(tc, y, x)
    nc.compile()
    return nc


def _in_maps(hs: np.ndarray):
    img = hs[0, IMG_START : IMG_START + IMG_LEN]
    maps = []
    for c in range(N_CORES):
        xc = img[TOK_PER_CORE * c : TOK_PER_CORE * (c + 1)]
        maps.append({"x": np.ascontiguousarray(xc)})
    return maps


def _host_runt(img: np.ndarray) -> np.ndarray:
    """Blend for patches 192-195 (the %16 runt the device skips): numpy."""
    iv = img.reshape(NUM_FRAMES // INTERVAL, INTERVAL, PATCHES, HIDDEN)
    runt = iv[:, :, 192:PATCHES, :]
    itok = runt[:, I_POS : I_POS + 1]
    d = itok.astype(np.float64) - runt.astype(np.float64)
    mask = np.abs(d).sum(-1) < THRESHOLD
    return np.where(mask[..., None], itok, runt).astype(np.float32)


def kernel(hidden_states: np.ndarray) -> np.ndarray:
    from concourse.bass_utils import run_bass_kernel_spmd

    hs = np.asarray(hidden_states, dtype=np.float32)
    assert hs.shape == (1, SEQ_LEN, HIDDEN), hs.shape
    nc = _build_nc()
    res = run_bass_kernel_spmd(nc, _in_maps(hs), list(range(N_CORES)))
    out = hs.copy()
    for c in range(N_CORES):
        out[0, IMG_START + TOK_PER_CORE * c : IMG_START + TOK_PER_CORE * (c + 1)] = (
            res.results[c]["y"]
        )
    img = hs[0, IMG_START : IMG_START + IMG_LEN]
    outv = out[0, IMG_START : IMG_START + IMG_LEN].reshape(
        NUM_FRAMES, PATCHES, HIDDEN
    )
    outv[:, 192:PATCHES, :] = _host_runt(img).reshape(NUM_FRAMES, 4, HIDDEN)
    return out


# revision 26
# speedup vs baseline: 2.7805x; 1.0830x over previous
"""CMC (Compressed Memory Compression) kernel for Trainium2 — 8 NeuronCores.

Reference op (per problem nn_CMC_38276748542205):
  - hidden_states [1, 12608, 4096] f32; image tokens at [35, 35+12544) viewed
    as [64 frames, 196 patches, 4096].
  - Frames form 16 intervals of 4; I-frame at position 3 of each interval.
  - SAD(token, I-frame token at same patch) over dim; mask = SAD < 1.12*4096.
  - Masked tokens replaced by the interval's I-frame token.

Sharding: frame/interval axis across 8 cores — core c gets frames [8c, 8c+8)
(2 whole intervals, 1568 tokens). Text tokens (64 rows) pass through on host.

Device kernel (per core, SPMD):
  - patch-major tiles [128 patches, 4 frames, 4096] via strided DMA; the
    I-frame is the f=3 slice of the same tile (no extra traffic, perfect
    partition alignment for the per-patch compare).
  - DVE: d_k = p3 - p_k (k in {0,1,2}; the f=3 output is the identity).
  - ACT: |d_k| with per-2048-chunk accumulation -> SAD (chunked so fp32
    summation error stays well below the min |SAD-thr| margin of ~0.034).
  - DVE: m = (sad < thr) as a per-partition 0/1 scalar, then the whole
    select/replace is ONE fused DVE op (scalar_tensor_tensor):
    out = (d * m) + p_k, written in place over p_k; 2-frame half-stores
    drain as soon as their frames are blended.
"""

import functools

import numpy as np

# ---- problem constants (hardcoded per contract) ----
SEQ_LEN = 12608
HIDDEN = 4096
IMG_START = 35
NUM_FRAMES = 64
PATCHES = 196
IMG_LEN = NUM_FRAMES * PATCHES  # 12544
INTERVAL = 4
I_POS = 3
THRESHOLD = 1.12 * HIDDEN  # 4587.52

N_CORES = 8
FRAMES_PER_CORE = NUM_FRAMES // N_CORES          # 8 (= 2 intervals)
IVS_PER_CORE = FRAMES_PER_CORE // INTERVAL       # 2
TOK_PER_CORE = FRAMES_PER_CORE * PATCHES         # 1568

SAD_CHUNK = 2048       # accumulation chunk for SAD numerical accuracy


def _kernel_body(tc, y_ap, x_ap):
    import concourse.bass as bass
    from concourse import mybir

    nc = tc.nc
    AF = mybir.ActivationFunctionType
    OP = mybir.AluOpType
    f32 = mybir.dt.float32

    xv = x_ap.rearrange("(f p) d -> p f d", f=FRAMES_PER_CORE, p=PATCHES)
    yv = y_ap.rearrange("(f p) d -> p f d", f=FRAMES_PER_CORE, p=PATCHES)

    import contextlib

    with contextlib.ExitStack() as ctx:
        p_pool = ctx.enter_context(tc.tile_pool(name="p", bufs=2))
        d_pool = ctx.enter_context(tc.tile_pool(name="d", bufs=3))
        abs_pool = ctx.enter_context(tc.tile_pool(name="absd", bufs=2))
        small_pool = ctx.enter_context(tc.tile_pool(name="small", bufs=12))

        n_sad_chunks = HIDDEN // SAD_CHUNK

        # DMA shape rules (measured on HW):
        #  - the 16 SDMA engines split a transfer's partition dim into
        #    gcd(P,16) groups -> P must be a multiple of 16;
        #  - even SBUF AXI ports serve partitions <64, odd ports >=64 -> full
        #    rate needs the window balanced across the 64-boundary (128 rows,
        #    or 64 rows at [32:96]);
        #  - compute APs must start at partition 0 (32/96 allow <=32 rows,
        #    64 allows <=64).
        # Patch coverage: chunk A = patches 0-127 at [0:128]; chunk B =
        # patches 128-191 at partitions [32:96] (compute on [0:96], the
        # garbage rows [0:32) are zeroed once and never stored). Patches
        # 192-195 (the %16 runt) are handled host-side in numpy.
        def compute_and_store(pt, q1, store):
            for k in (2, 0, 1):  # f=3 (I-frame) passes through untouched
                d_t = d_pool.tile([128, HIDDEN], f32)
                nc.vector.tensor_tensor(
                    d_t[:q1, :],
                    pt[:q1, I_POS, :],
                    pt[:q1, k, :],
                    op=OP.subtract,
                )
                sadp = small_pool.tile([128, n_sad_chunks], f32, tag="sadp")
                for h in range(n_sad_chunks):
                    ab = abs_pool.tile([128, SAD_CHUNK], f32)
                    nc.scalar.activation(
                        ab[:q1, :],
                        d_t[:q1, bass.ts(h, SAD_CHUNK)],
                        AF.Abs,
                        accum_out=sadp[:q1, h : h + 1],
                    )
                m_t = small_pool.tile([128, 1], f32, tag="m")
                # fused: m = (sadp0 + sadp1) < thr — both scalars per-partition
                nc.vector.tensor_scalar(
                    m_t[:q1, :],
                    sadp[:q1, 0:1],
                    sadp[:q1, 1:2],
                    float(THRESHOLD),
                    op0=OP.add,
                    op1=OP.is_lt,
                )
                # fused blend: out = (d * m) + p_k, in place over p_k
                nc.vector.scalar_tensor_tensor(
                    pt[:q1, k, :],
                    d_t[:q1, :],
                    m_t[:q1, :],
                    pt[:q1, k, :],
                    op0=OP.mult,
                    op1=OP.add,
                )
                if k == 2:
                    store(2, INTERVAL)  # frames 2-3 ready: drain early
            store(0, 2)

        for iv in range(IVS_PER_CORE):
            f0 = iv * INTERVAL

            # ---- chunk A: patches 0-127 at [0:128] ----
            ptA = p_pool.tile([128, INTERVAL, HIDDEN], f32, tag="pt")
            # paired loads, I-frame half first (sub k=2 needs only f2/f3)
            nc.sync.dma_start(ptA[:, 2:4, :], xv[0:128, f0 + 2 : f0 + 4, :])
            nc.sync.dma_start(ptA[:, 0:2, :], xv[0:128, f0 : f0 + 2, :])

            def store_a(fa, fb, ptA=ptA, f0=f0):
                # stores ride the ACT HWDGE ring so load/store descriptor
                # streams interleave instead of sharing one FIFO
                nc.scalar.dma_start(
                    yv[0:128, f0 + fa : f0 + fb, :], ptA[:, fa:fb, :]
                )

            compute_and_store(ptA, 128, store_a)

            # ---- chunk B: patches 128-191 at partitions [32:96] ----
            ptB = p_pool.tile([128, INTERVAL, HIDDEN], f32, tag="pt")
            # rows [0:32) are read by the [0:96] compute ops but never
            # loaded; zero them (gpsimd, off the critical engines)
            nc.gpsimd.memset(ptB[0:32, :, :], 0.0)
            nc.sync.dma_start(
                ptB[32:96, 2:4, :], xv[128:192, f0 + 2 : f0 + 4, :]
            )
            nc.sync.dma_start(ptB[32:96, 0:2, :], xv[128:192, f0 : f0 + 2, :])

            def store_b(fa, fb, ptB=ptB, f0=f0):
                nc.scalar.dma_start(
                    yv[128:192, f0 + fa : f0 + fb, :], ptB[32:96, fa:fb, :]
                )

            compute_and_store(ptB, 96, store_b)


@functools.cache
def _build_nc():
    import concourse.bacc as bacc
    import concourse.tile as tile
    from concourse import mybir

    nc = bacc.Bacc(
        "TRN2",
        target_bir_lowering=False,
        debug=False,
        enable_asserts=False,
        num_devices=N_CORES,
    )
    x = nc.dram_tensor(
        "x", [TOK_PER_CORE, HIDDEN], mybir.dt.float32, kind="ExternalInput"
    ).ap()
    y = nc.dram_tensor(
        "y", [TOK_PER_CORE, HIDDEN], mybir.dt.float32, kind="ExternalOutput"
    ).ap()
    with tile.TileContext(nc) as tc:
        _kernel_body(tc, y, x)
    nc.compile()
    return nc


def _in_maps(hs: np.ndarray):
    img = hs[0, IMG_START : IMG_START + IMG_LEN]
    maps = []
    for c in range(N_CORES):
        xc = img[TOK_PER_CORE * c : TOK_PER_CORE * (c + 1)]
        maps.append({"x": np.ascontiguousarray(xc)})
    return maps


def _host_runt(img: np.ndarray) -> np.ndarray:
    """Blend for patches 192-195 (the %16 runt the device skips): numpy."""
    iv = img.reshape(NUM_FRAMES // INTERVAL, INTERVAL, PATCHES, HIDDEN)
    runt = iv[:, :, 192:PATCHES, :]
    itok = runt[:, I_POS : I_POS + 1]
    d = itok.astype(np.float64) - runt.astype(np.float64)
    mask = np.abs(d).sum(-1) < THRESHOLD
    return np.where(mask[..., None], itok, runt).astype(np.float32)


def kernel(hidden_states: np.ndarray) -> np.ndarray:
    from concourse.bass_utils import run_bass_kernel_spmd

    hs = np.asarray(hidden_states, dtype=np.float32)
    assert hs.shape == (1, SEQ_LEN, HIDDEN), hs.shape
    nc = _build_nc()
    res = run_bass_kernel_spmd(nc, _in_maps(hs), list(range(N_CORES)))
    out = hs.copy()
    for c in range(N_CORES):
        out[0, IMG_START + TOK_PER_CORE * c : IMG_START + TOK_PER_CORE * (c + 1)] = (
            res.results[c]["y"]
        )
    img = hs[0, IMG_START : IMG_START + IMG_LEN]
    outv = out[0, IMG_START : IMG_START + IMG_LEN].reshape(
        NUM_FRAMES, PATCHES, HIDDEN
    )
    outv[:, 192:PATCHES, :] = _host_runt(img).reshape(NUM_FRAMES, 4, HIDDEN)
    return out
